# revision 21
# baseline (speedup 1.0000x reference)
"""Trainium2 Bass kernel for nn_BoundaryConsistencyLoss.

loss = mean-over-valid-windows of mean-over-batch (pvar - tvar)^2 where
pvar/tvar are masked variances of sigmoid-probs / targets over sliding
windows of 5 along L.

Strategy: pure data parallel over batch (512 = 8 cores x 64 rows).
Per core, SBUF layout [128 partitions = 2 L-halves x 64 batch rows,
free = L-chunk].

Math: with m=mask, t=targets, p=sigmoid(x1-x0), z=(t AND m)+m = m+t*m
(so m=min(z,1), tm=relu(z-1)), define windowed sums via fused
cumsum-custom-ops (one DVE instruction each):
  c_m = cumsum(m), c_G = cumsum(p^2 m - tm), c_H = cumsum(pm - tm),
  c_K = cumsum(pm + tm)
then per window j: X_w[j] = c_X[j+5]-c_X[j], and
  diff = pvar - tvar = r*G_w - r^2*H_w*K_w,  r = 1/(msum+eps)
  d2 = (r*(G_w - r*H_w*K_w))^2
For empty windows (msum=0) all of G_w/H_w/K_w are exactly 0, so d2=0
regardless of r: the clamp max(msum,1) is replaced by a tiny eps bias
inside the reciprocal, and invalid windows self-gate out of the sum.
The total sum of d2 per partition is accumulated for free by the DVE
accum port on the final squaring op; only the per-window validity
indicator (batch sum of K_w, >0 iff the reference's msum total is >0)
goes through a ones-matmul on the tensor engine.  Host sums the 8
cores' partials and finishes the tiny reduction exactly like the
reference.

Engine budget per chunk (the shared SBUF port between DVE-src1 and
GpSimd is the scarce resource): gpsimd runs ONLY SWDGE descriptor
generation; z is assembled during the DMA itself with CCE accum ops
(bypass/min/add over t,m,m); all elementwise work rides the vector
engine; scalar does sigmoid/recip/psum-evac; tensor does the validity
matmul.
"""

import sys

if "/opt/trn_rl_repo" not in sys.path:
    sys.path.insert(0, "/opt/trn_rl_repo")

import numpy as np

import concourse.bass as bass
import concourse.tile as tile
from concourse import bacc, dve_ops, mybir
from concourse.bass_interp import get_hw_module
from concourse.bass_utils import run_bass_kernel_spmd
from concourse.dve_spec import (
    AluOp,
    One,
    Spec,
    Src0,
    Src1,
    _has_src1,
    lower,
    minn,
    relu,
    scan,
    sq,
)
from concourse.dve_uop import DveOpSpec

F32 = mybir.dt.float32
BF16 = mybir.dt.bfloat16
I32 = mybir.dt.int32
AF = mybir.ActivationFunctionType
OP = mybir.AluOpType

NCORES = 8
B, L, C = 512, 16384, 2
BL = B // NCORES          # 64 batch rows per core
LH = L // 2               # 8192: per-half length
W = 5
NW = L - W + 1            # 16380 windows
P = 128

CK = 1024                 # windows computed per chunk
CKH = CK + (W - 1)        # data elements per chunk (halo 4)
NCH = LH // CK
CP = CKH + 2              # c-tile page stride (col 0 is an explicit zero;
                          # even stride keeps the windowed-diff reads 8B-aligned
                          # so the DVE runs them in 2x mode)

R_EPS = float(2.0 ** -30)


# --------------------------------------------------------------------------
# custom DVE ops (registered at runtime; sha computed the same way
# DveOp.compile does, so the golden check passes)
# --------------------------------------------------------------------------
def _register_op(name, spec, subdim=False):
    for op in dve_ops.OPS:
        if op.name == name:
            return op
    opcode = dve_ops._CUSTOM_DVE_ROW_BASE + len(dve_ops.OPS)
    shas = {}
    for ver in ("v3", "v4"):
        s = DveOpSpec(
            name=name, opcode=opcode, uops=lower(spec, ver=ver), rd1_en=_has_src1(spec)
        )
        shas[ver] = s.sha(ver)
    op = dve_ops.DveOp(name, spec, subdim=subdim, uops_sha=shas)
    dve_ops.OPS.append(op)
    dve_ops._SUB_OPCODE_FOR_NAME[name] = opcode
    dve_ops.CUSTOM_DVE_SPECS[name] = spec
    return op


def _f32(a):
    return np.asarray(a, np.float32)


def _z_parts(z):
    z = _f32(z)
    return np.minimum(z, 1.0), np.maximum(z - 1.0, 0.0)


def _ref_mscan(in0, in1, s0, s1, imm2):
    return np.cumsum(np.minimum(_f32(in0), 1.0), axis=-1, dtype=np.float32)


def _ref_gscan(in0, in1, s0, s1, imm2):
    m, tm = _z_parts(in1)
    return np.cumsum(_f32(in0) * _f32(in0) * m - tm, axis=-1, dtype=np.float32)


def _ref_hscan(in0, in1, s0, s1, imm2):
    m, tm = _z_parts(in1)
    return np.cumsum(_f32(in0) * m - tm, axis=-1, dtype=np.float32)


def _ref_kscan(in0, in1, s0, s1, imm2):
    m, tm = _z_parts(in1)
    return np.cumsum(_f32(in0) * m + tm, axis=-1, dtype=np.float32)


_m_of_z = minn(Src1, One)
_tm_of_z = relu(Src1 - One)

MSCAN = _register_op(
    "BC2_MSCAN", Spec(body=scan(AluOp.ADD, minn(Src0, One)), reference=_ref_mscan)
)
GSCAN = _register_op(
    "BC2_GSCAN",
    Spec(body=scan(AluOp.ADD, sq(Src0) * _m_of_z - _tm_of_z), reference=_ref_gscan),
)
HSCAN = _register_op(
    "BC2_HSCAN",
    Spec(body=scan(AluOp.ADD, Src0 * _m_of_z - _tm_of_z), reference=_ref_hscan),
)
KSCAN = _register_op(
    "BC2_KSCAN",
    Spec(body=scan(AluOp.ADD, Src0 * _m_of_z + _tm_of_z), reference=_ref_kscan),
)

# d2 = (in0 * in1)^2, with a free running per-partition sum on the accum port
SQMULA = _register_op(
    "BC2_SQMULA",
    Spec(
        body=sq(Src0 * Src1),
        accum=AluOp.ADD,
        reference=lambda in0, in1, s0, s1, imm2: (_f32(in0) * _f32(in1)) ** 2,
    ),
)

# z' = min(t, m) + m fallback (single fused op) if the DMA-accum path is off
ZPRIME = _register_op(
    "BC2_ZPRIME",
    Spec(
        body=minn(Src0, Src1) + Src1,
        reference=lambda in0, in1, s0, s1, imm2: np.minimum(_f32(in0), _f32(in1))
        + _f32(in1),
    ),
)


def _split_sync_waits(nc, max_waits=1):
    """walrus TPB_CTRL codegen rejects >1 explicit sem wait on Drain-class
    instructions; move excess waits onto preceding same-engine no-ops."""
    for fn in nc.m.functions:
        for bb in fn.blocks:
            new_insts = []
            for ins in bb.instructions:
                si = getattr(ins, "sync_info", None)
                waits = list(si.on_wait) if si is not None else []
                if len(waits) > max_waits:
                    extra, keep = waits[:-max_waits], waits[-max_waits:]
                    for j in range(0, len(extra), max_waits):
                        new_insts.append(
                            mybir.InstNoOp(
                                name=f"{ins.name}_wsplit{j}",
                                engine=ins.engine,
                                ins=[],
                                outs=[],
                                sync_info=mybir.SyncInfo(
                                    on_wait=extra[j : j + max_waits], on_update=[]
                                ),
                            )
                        )
                    si.on_wait.clear()
                    si.on_wait.extend(keep)
                new_insts.append(ins)
            bb.instructions = new_insts


CFG = {
    "dsg_engine": "vector",
    "prefetch": 1,         # chunks of DMA issued ahead; deeper prefetch HURTS:
                           # queued SWDGE DMAs round-robin at packet granularity,
                           # so a deep queue delays the oldest chunk's completion
}


def _act_scalar(nc, out, in_, func, bias=0.0, scale=1.0):
    """Direct InstActivation emit (bass blocks AF.Reciprocal behind a
    ValueError; our recip inputs are small ints plus eps, well within the
    2e-2 budget)."""
    eng = nc.scalar
    ins = [eng.lower_ap(in_)]
    for val in (bias, scale, 0.0):  # bias, scale, alpha
        ins.append(mybir.ImmediateValue(dtype=mybir.dt.float32, value=val))
    return eng.add_instruction(
        mybir.InstActivation(
            name=nc.get_next_instruction_name(),
            func=func,
            ins=ins,
            outs=[eng.lower_ap(out)],
        )
    )


def _emit_body(nc, pools, ones_bf, accT, preds, tg, mk, mst_o):
    io, mid, wsp, cmb, ps = pools

    def pv(col0, ncols):
        a = preds[:, :, :]
        return bass.AP(tensor=a.tensor, offset=col0, ap=[[L, 2], [2 * L, BL], [1, ncols]])

    def iv(t, col0, ncols):
        a = t[:, :]
        return bass.AP(tensor=a.tensor, offset=col0, ap=[[LH, 2], [L, BL], [1, ncols]])

    pflat = preds.rearrange("b l c -> b (l c)")  # [64, 2*L]

    xps, zts = {}, {}

    def load_chunk(c):
        # ti/mi first: their consumers (ZPRIME, MSCAN) head the chunk's
        # dependency chain, and same-queue SWDGE packets round-robin, so
        # issue order is completion order
        main_z = CK if c == NCH - 1 else CKH
        ti = io.tile([P, CKH], I32, tag="ti")
        mi = io.tile([P, CKH], I32, tag="mi")
        zts[c] = (ti, mi)
        nc.gpsimd.dma_start(out=ti[:, :main_z], in_=iv(tg, c * CK, main_z))
        nc.gpsimd.dma_start(out=mi[:, :main_z], in_=iv(mk, c * CK, main_z))
        # predictions piece [128, 2*CKH] f32 (1 MiB + halo)
        xp = io.tile([P, 2 * CKH], F32, tag="xp")
        xps[c] = xp
        main_p = 2 * CK if c == NCH - 1 else 2 * CKH
        nc.gpsimd.dma_start(out=xp[:, :main_p], in_=pv(2 * c * CK, main_p))
        if c == NCH - 1:
            # h=0 rows wrap into the start of the second half; h=1 rows are
            # past the end of L and read as zero
            nc.sync.dma_start(out=xp[0:64, 2 * CK :], in_=pflat[:, 2 * LH : 2 * LH + 8])
            nc.vector.memset(xp[64:128, 2 * CK :], 0.0)
            nc.scalar.dma_start(out=ti[0:64, CK:], in_=tg[:, LH : LH + 4])
            nc.sync.dma_start(out=mi[0:64, CK:], in_=mk[:, LH : LH + 4])
            nc.vector.memset(ti[64:128, CK:], 0)
            nc.vector.memset(mi[64:128, CK:], 0)

    for c in range(min(CFG["prefetch"], NCH)):
        load_chunk(c)

    for c in range(NCH):
        if c + CFG["prefetch"] < NCH:
            load_chunk(c + CFG["prefetch"])
        xp = xps.pop(c)
        ti, mi = zts.pop(c)
        zt = mid.tile([P, CKH], F32, tag="zt")
        nc.vector._custom_dve(ZPRIME, out=zt[:, :], in0=ti[:, :], in1=mi[:, :])

        xvv = xp.rearrange("p (l two) -> p l two", two=2)
        dsg = mid.tile([P, CKH], F32, tag="dsg")
        getattr(nc, CFG["dsg_engine"]).tensor_sub(
            dsg[:, :], xvv[:, :, 1], xvv[:, :, 0]
        )
        pp = mid.tile([P, CKH], F32, tag="pp")
        nc.scalar.activation(pp[:, :], dsg[:, :], AF.Sigmoid)

        # fused cumsums into one 4-page mega-tile; col 0 of each page is an
        # explicit zero so X_w[j] = c[j+5] - c[j] holds for j in [0, CK)
        c4 = wsp.tile([P, 4 * CP], F32, tag="c4")
        c4v = c4.rearrange("p (s k) -> p s k", s=4)
        nc.vector.memset(c4v[:, :, 0:1], 0.0)
        nc.vector._custom_dve(MSCAN, out=c4[:, 1 : 1 + CKH], in0=mi[:, :])
        for i, op_ in ((1, GSCAN), (2, HSCAN), (3, KSCAN)):
            nc.vector._custom_dve(
                op_,
                out=c4[:, i * CP + 1 : i * CP + 1 + CKH],
                in0=pp[:, :],
                in1=zt[:, :],
            )

        # windowed diffs, one plain 2D sub per stream (3D paged APs drop the
        # DVE to 1x; separate 2D subs run at 2x) -> bf16 [128, 4, CK]
        w4 = cmb.tile([P, 4 * CK], BF16, tag="w4")
        w4v = w4.rearrange("p (s k) -> p s k", s=4)
        for i in range(4):
            nc.vector.tensor_sub(
                w4v[:, i, :], c4v[:, i, 5 : 5 + CK], c4v[:, i, 0:CK]
            )

        # r = 1/(msum + eps) on the idle Act engine, bf16 out
        r = cmb.tile([P, CK], BF16, tag="r")
        _act_scalar(nc, r[:, :], w4v[:, 0, :], AF.Reciprocal, bias=R_EPS)

        V = cmb.tile([P, CK], BF16, tag="V")
        nc.vector.tensor_mul(V[:, :], w4v[:, 2, :], w4v[:, 3, :])
        V2 = cmb.tile([P, CK], BF16, tag="V2")
        nc.vector.tensor_mul(V2[:, :], V[:, :], r[:, :])
        U = cmb.tile([P, CK], BF16, tag="U")
        nc.vector.tensor_sub(U[:, :], w4v[:, 1, :], V2[:, :])
        Y = cmb.tile([P, CK], BF16, tag="Y")
        nc.vector.tensor_mul(Y[:, :], U[:, :], r[:, :])
        # d2 = Y^2 with the batch-partial sum riding the Act accum port
        d2 = cmb.tile([P, CK], BF16, tag="d2")
        nc.scalar.activation(
            d2[:, :], Y[:, :], AF.Square, accum_out=accT[:, c : c + 1]
        )

        # validity partial: batch sum of K_w per half (>0 iff ref msum-total >0)
        mst_ps = ps.tile([2, CK], F32, tag="mstp")
        for q in range(CK // 512):
            nc.tensor.matmul(
                mst_ps[:, q * 512 : (q + 1) * 512],
                ones_bf[:, :],
                w4v[:, 3, q * 512 : (q + 1) * 512],
                start=True,
                stop=True,
            )
        mst_ev = cmb.tile([2, CK], F32, tag="mst_ev")
        nc.scalar.copy(mst_ev[:, :], mst_ps[:, :])
        nc.sync.dma_start(out=mst_o[:, c * CK : (c + 1) * CK], in_=mst_ev[:, :])


def _build_program():
    nc = bacc.Bacc(
        "TRN2",
        target_bir_lowering=False,
        debug=False,
        enable_asserts=False,
        num_devices=NCORES,
    )
    preds = nc.dram_tensor("predictions", [BL, L, C], F32, kind="ExternalInput")
    tg = nc.dram_tensor("targets", [BL, L], I32, kind="ExternalInput")
    mk = nc.dram_tensor("mask", [BL, L], I32, kind="ExternalInput")
    mst_o = nc.dram_tensor("mst", [2, LH], F32, kind="ExternalOutput")
    acc_o = nc.dram_tensor("acc", [P, NCH], F32, kind="ExternalOutput")

    with tile.TileContext(nc) as tc:
        with (
            tc.tile_pool(name="io", bufs=CFG["prefetch"] + 1) as io,
            tc.tile_pool(name="mid", bufs=3) as mid,
            tc.tile_pool(name="wsp", bufs=3) as wsp,
            tc.tile_pool(name="cmb", bufs=3) as cmb,
            tc.tile_pool(name="const", bufs=1) as const,
            tc.tile_pool(name="ps", bufs=2, space="PSUM") as ps,
        ):
            ones_bf = const.tile([P, 2], BF16)
            nc.vector.memset(ones_bf[:, :], 0.0)
            nc.vector.memset(ones_bf[0:64, 0:1], 1.0)
            nc.vector.memset(ones_bf[64:128, 1:2], 1.0)
            accT = const.tile([P, NCH], F32)

            pools = (io, mid, wsp, cmb, ps)
            _emit_body(nc, pools, ones_bf, accT, preds, tg, mk, mst_o)
            nc.sync.dma_start(out=acc_o[:, :], in_=accT[:, :])

    nc.compile()
    nc.m = get_hw_module(nc.m)
    _split_sync_waits(nc)
    return nc


_NC_CACHE = {}


def _get_nc():
    if "nc" not in _NC_CACHE:
        _NC_CACHE["nc"] = _build_program()
    return _NC_CACHE["nc"]


def run_on_device(predictions, targets, mask, **spmd_kwargs):
    """Shard inputs, run the Bass kernel on 8 cores."""
    nc = _get_nc()
    predictions = np.ascontiguousarray(np.asarray(predictions, np.float32))
    targets = np.ascontiguousarray(np.asarray(targets, np.int32))
    mask = np.ascontiguousarray(np.asarray(mask, np.int32))
    in_maps = []
    for i in range(NCORES):
        sl = slice(i * BL, (i + 1) * BL)
        in_maps.append(
            {
                "predictions": np.ascontiguousarray(predictions[sl]),
                "targets": np.ascontiguousarray(targets[sl]),
                "mask": np.ascontiguousarray(mask[sl]),
            }
        )
    res = run_bass_kernel_spmd(nc, in_maps, core_ids=list(range(NCORES)), **spmd_kwargs)
    return res


def combine_host(results):
    ssd_sum = 0.0
    mst_tot = np.zeros(NW, np.float64)
    for out in results:
        ssd_sum += float(out["acc"].astype(np.float64).sum())
        mst = out["mst"]
        mst_tot += np.concatenate([mst[0], mst[1][: NW - LH]])
    valid = (mst_tot > 0).astype(np.float64)
    cnt = max(valid.sum(), 1.0)
    loss = ssd_sum / B / cnt
    return np.asarray(loss, dtype=np.float32)


def kernel(predictions, targets, mask):
    res = run_on_device(predictions, targets, mask)
    return combine_host(res.results)


if __name__ == "__main__":
    rng = np.random.default_rng(0)
    p = rng.standard_normal((B, L, C), dtype=np.float32)
    t = rng.integers(0, 2, (B, L)).astype(np.int32)
    m = rng.integers(0, 2, (B, L)).astype(np.int32)
    print(kernel(p, t, m))


# revision 23
# speedup vs baseline: 1.0413x; 1.0413x over previous
"""Trainium2 Bass kernel for nn_BoundaryConsistencyLoss.

loss = mean-over-valid-windows of mean-over-batch (pvar - tvar)^2 where
pvar/tvar are masked variances of sigmoid-probs / targets over sliding
windows of 5 along L.

Strategy: pure data parallel over batch (512 = 8 cores x 64 rows).
Per core, SBUF layout [128 partitions = 2 L-halves x 64 batch rows,
free = L-chunk].

Math: with m=mask, t=targets, p=sigmoid(x1-x0), z=(t AND m)+m = m+t*m
(so m=min(z,1), tm=relu(z-1)), define windowed sums via fused
cumsum-custom-ops (one DVE instruction each):
  c_m = cumsum(m), c_G = cumsum(p^2 m - tm), c_H = cumsum(pm - tm),
  c_K = cumsum(pm + tm)
then per window j: X_w[j] = c_X[j+5]-c_X[j], and
  diff = pvar - tvar = r*G_w - r^2*H_w*K_w,  r = 1/(msum+eps)
  d2 = (r*(G_w - r*H_w*K_w))^2
For empty windows (msum=0) all of G_w/H_w/K_w are exactly 0, so d2=0
regardless of r: the clamp max(msum,1) is replaced by a tiny eps bias
inside the reciprocal, and invalid windows self-gate out of the sum.
The total sum of d2 per partition is accumulated for free by the DVE
accum port on the final squaring op; only the per-window validity
indicator (batch sum of K_w, >0 iff the reference's msum total is >0)
goes through a ones-matmul on the tensor engine.  Host sums the 8
cores' partials and finishes the tiny reduction exactly like the
reference.

Engine budget per chunk (the shared SBUF port between DVE-src1 and
GpSimd is the scarce resource): gpsimd runs ONLY SWDGE descriptor
generation; z is assembled during the DMA itself with CCE accum ops
(bypass/min/add over t,m,m); all elementwise work rides the vector
engine; scalar does sigmoid/recip/psum-evac; tensor does the validity
matmul.
"""

import sys

if "/opt/trn_rl_repo" not in sys.path:
    sys.path.insert(0, "/opt/trn_rl_repo")

import numpy as np

import concourse.bass as bass
import concourse.tile as tile
from concourse import bacc, dve_ops, mybir
from concourse.bass_interp import get_hw_module
from concourse.bass_utils import run_bass_kernel_spmd
from concourse.dve_spec import (
    AluOp,
    One,
    Spec,
    Src0,
    Src1,
    _has_src1,
    lower,
    minn,
    relu,
    scan,
    sq,
)
from concourse.dve_uop import DveOpSpec

F32 = mybir.dt.float32
BF16 = mybir.dt.bfloat16
I32 = mybir.dt.int32
AF = mybir.ActivationFunctionType
OP = mybir.AluOpType

NCORES = 8
B, L, C = 512, 16384, 2
BL = B // NCORES          # 64 batch rows per core
LH = L // 2               # 8192: per-half length
W = 5
NW = L - W + 1            # 16380 windows
P = 128

CK = 1024                 # windows computed per chunk
CKH = CK + (W - 1)        # data elements per chunk (halo 4)
NCH = LH // CK
CP = CKH + 4              # c-tile page stride (col 0 is an explicit zero;
                          # 32B-aligned pages keep the windowed-diff reads in
                          # the DVE's 2x dual-port mode)

R_EPS = float(2.0 ** -30)


# --------------------------------------------------------------------------
# custom DVE ops (registered at runtime; sha computed the same way
# DveOp.compile does, so the golden check passes)
# --------------------------------------------------------------------------
def _register_op(name, spec, subdim=False):
    for op in dve_ops.OPS:
        if op.name == name:
            return op
    opcode = dve_ops._CUSTOM_DVE_ROW_BASE + len(dve_ops.OPS)
    shas = {}
    for ver in ("v3", "v4"):
        s = DveOpSpec(
            name=name, opcode=opcode, uops=lower(spec, ver=ver), rd1_en=_has_src1(spec)
        )
        shas[ver] = s.sha(ver)
    op = dve_ops.DveOp(name, spec, subdim=subdim, uops_sha=shas)
    dve_ops.OPS.append(op)
    dve_ops._SUB_OPCODE_FOR_NAME[name] = opcode
    dve_ops.CUSTOM_DVE_SPECS[name] = spec
    return op


def _f32(a):
    return np.asarray(a, np.float32)


def _z_parts(z):
    z = _f32(z)
    return np.minimum(z, 1.0), np.maximum(z - 1.0, 0.0)


def _ref_mscan(in0, in1, s0, s1, imm2):
    return np.cumsum(np.minimum(_f32(in0), 1.0), axis=-1, dtype=np.float32)


def _ref_gscan(in0, in1, s0, s1, imm2):
    m, tm = _z_parts(in1)
    return np.cumsum(_f32(in0) * _f32(in0) * m - tm, axis=-1, dtype=np.float32)


def _ref_hscan(in0, in1, s0, s1, imm2):
    m, tm = _z_parts(in1)
    return np.cumsum(_f32(in0) * m - tm, axis=-1, dtype=np.float32)


def _ref_kscan(in0, in1, s0, s1, imm2):
    m, tm = _z_parts(in1)
    return np.cumsum(_f32(in0) * m + tm, axis=-1, dtype=np.float32)


_m_of_z = minn(Src1, One)
_tm_of_z = relu(Src1 - One)

MSCAN = _register_op(
    "BC2_MSCAN", Spec(body=scan(AluOp.ADD, minn(Src0, One)), reference=_ref_mscan)
)
GSCAN = _register_op(
    "BC2_GSCAN",
    Spec(body=scan(AluOp.ADD, sq(Src0) * _m_of_z - _tm_of_z), reference=_ref_gscan),
)
HSCAN = _register_op(
    "BC2_HSCAN",
    Spec(body=scan(AluOp.ADD, Src0 * _m_of_z - _tm_of_z), reference=_ref_hscan),
)
KSCAN = _register_op(
    "BC2_KSCAN",
    Spec(body=scan(AluOp.ADD, Src0 * _m_of_z + _tm_of_z), reference=_ref_kscan),
)

# d2 = (in0 * in1)^2, with a free running per-partition sum on the accum port
SQMULA = _register_op(
    "BC2_SQMULA",
    Spec(
        body=sq(Src0 * Src1),
        accum=AluOp.ADD,
        reference=lambda in0, in1, s0, s1, imm2: (_f32(in0) * _f32(in1)) ** 2,
    ),
)

# z' = min(t, m) + m fallback (single fused op) if the DMA-accum path is off
ZPRIME = _register_op(
    "BC2_ZPRIME",
    Spec(
        body=minn(Src0, Src1) + Src1,
        reference=lambda in0, in1, s0, s1, imm2: np.minimum(_f32(in0), _f32(in1))
        + _f32(in1),
    ),
)


def _split_sync_waits(nc, max_waits=1):
    """walrus TPB_CTRL codegen rejects >1 explicit sem wait on Drain-class
    instructions; move excess waits onto preceding same-engine no-ops."""
    for fn in nc.m.functions:
        for bb in fn.blocks:
            new_insts = []
            for ins in bb.instructions:
                si = getattr(ins, "sync_info", None)
                waits = list(si.on_wait) if si is not None else []
                if len(waits) > max_waits:
                    extra, keep = waits[:-max_waits], waits[-max_waits:]
                    for j in range(0, len(extra), max_waits):
                        new_insts.append(
                            mybir.InstNoOp(
                                name=f"{ins.name}_wsplit{j}",
                                engine=ins.engine,
                                ins=[],
                                outs=[],
                                sync_info=mybir.SyncInfo(
                                    on_wait=extra[j : j + max_waits], on_update=[]
                                ),
                            )
                        )
                    si.on_wait.clear()
                    si.on_wait.extend(keep)
                new_insts.append(ins)
            bb.instructions = new_insts


CFG = {
    "dsg_engine": "vector",
    "prefetch": 2,         # chunks of DMA issued ahead; deeper prefetch HURTS:
                           # queued SWDGE DMAs round-robin at packet granularity,
                           # so a deep queue delays the oldest chunk's completion
}


def _act_scalar(nc, out, in_, func, bias=0.0, scale=1.0):
    """Direct InstActivation emit (bass blocks AF.Reciprocal behind a
    ValueError; our recip inputs are small ints plus eps, well within the
    2e-2 budget)."""
    eng = nc.scalar
    ins = [eng.lower_ap(in_)]
    for val in (bias, scale, 0.0):  # bias, scale, alpha
        ins.append(mybir.ImmediateValue(dtype=mybir.dt.float32, value=val))
    return eng.add_instruction(
        mybir.InstActivation(
            name=nc.get_next_instruction_name(),
            func=func,
            ins=ins,
            outs=[eng.lower_ap(out)],
        )
    )


def _emit_body(nc, pools, ones_bf, accT, preds, tg, mk, mst_o):
    io, mid, wsp, cmb, ps = pools

    def pv(col0, ncols):
        a = preds[:, :, :]
        return bass.AP(tensor=a.tensor, offset=col0, ap=[[L, 2], [2 * L, BL], [1, ncols]])

    def iv(t, col0, ncols):
        a = t[:, :]
        return bass.AP(tensor=a.tensor, offset=col0, ap=[[LH, 2], [L, BL], [1, ncols]])

    pflat = preds.rearrange("b l c -> b (l c)")  # [64, 2*L]

    xps, zts = {}, {}

    def load_chunk(c):
        # ti/mi first: their consumers (ZPRIME, MSCAN) head the chunk's
        # dependency chain, and same-queue SWDGE packets round-robin, so
        # issue order is completion order
        main_z = CK if c == NCH - 1 else CKH
        ti = io.tile([P, CKH], I32, tag="ti")
        mi = io.tile([P, CKH], I32, tag="mi")
        zts[c] = (ti, mi)
        nc.gpsimd.dma_start(out=ti[:, :main_z], in_=iv(tg, c * CK, main_z))
        nc.gpsimd.dma_start(out=mi[:, :main_z], in_=iv(mk, c * CK, main_z))
        # predictions piece [128, 2*CKH] f32 (1 MiB + halo)
        xp = io.tile([P, 2 * CKH], F32, tag="xp")
        xps[c] = xp
        main_p = 2 * CK if c == NCH - 1 else 2 * CKH
        nc.gpsimd.dma_start(out=xp[:, :main_p], in_=pv(2 * c * CK, main_p))
        if c == NCH - 1:
            # h=0 rows wrap into the start of the second half; h=1 rows are
            # past the end of L and read as zero
            nc.sync.dma_start(out=xp[0:64, 2 * CK :], in_=pflat[:, 2 * LH : 2 * LH + 8])
            nc.vector.memset(xp[64:128, 2 * CK :], 0.0)
            nc.scalar.dma_start(out=ti[0:64, CK:], in_=tg[:, LH : LH + 4])
            nc.sync.dma_start(out=mi[0:64, CK:], in_=mk[:, LH : LH + 4])
            nc.vector.memset(ti[64:128, CK:], 0)
            nc.vector.memset(mi[64:128, CK:], 0)

    for c in range(min(CFG["prefetch"], NCH)):
        load_chunk(c)

    for c in range(NCH):
        if c + CFG["prefetch"] < NCH:
            load_chunk(c + CFG["prefetch"])
        xp = xps.pop(c)
        ti, mi = zts.pop(c)
        zt = mid.tile([P, CKH], F32, tag="zt")
        nc.vector._custom_dve(ZPRIME, out=zt[:, :], in0=ti[:, :], in1=mi[:, :])

        xvv = xp.rearrange("p (l two) -> p l two", two=2)
        dsg = mid.tile([P, CKH], F32, tag="dsg")
        getattr(nc, CFG["dsg_engine"]).tensor_sub(
            dsg[:, :], xvv[:, :, 1], xvv[:, :, 0]
        )
        pp = mid.tile([P, CKH], F32, tag="pp")
        nc.scalar.activation(pp[:, :], dsg[:, :], AF.Sigmoid)

        # fused cumsums into one 4-page mega-tile; col 0 of each page is an
        # explicit zero so X_w[j] = c[j+5] - c[j] holds for j in [0, CK)
        c4 = wsp.tile([P, 4 * CP], F32, tag="c4")
        c4v = c4.rearrange("p (s k) -> p s k", s=4)
        nc.vector.memset(c4v[:, :, 0:1], 0.0)
        nc.vector._custom_dve(MSCAN, out=c4[:, 1 : 1 + CKH], in0=mi[:, :])
        for i, op_ in ((1, GSCAN), (2, HSCAN), (3, KSCAN)):
            nc.vector._custom_dve(
                op_,
                out=c4[:, i * CP + 1 : i * CP + 1 + CKH],
                in0=pp[:, :],
                in1=zt[:, :],
            )

        # windowed diffs, one plain 2D sub per stream (3D paged APs drop the
        # DVE to 1x; separate 2D subs run at 2x) -> bf16 [128, 4, CK]
        w4 = cmb.tile([P, 4 * CK], BF16, tag="w4")
        w4v = w4.rearrange("p (s k) -> p s k", s=4)
        for i in range(4):
            nc.vector.tensor_sub(
                w4v[:, i, :], c4v[:, i, 5 : 5 + CK], c4v[:, i, 0:CK]
            )

        # r = 1/(msum + eps) on the idle Act engine, bf16 out
        r = cmb.tile([P, CK], BF16, tag="r")
        _act_scalar(nc, r[:, :], w4v[:, 0, :], AF.Reciprocal, bias=R_EPS)

        V = cmb.tile([P, CK], BF16, tag="V")
        nc.vector.tensor_mul(V[:, :], w4v[:, 2, :], w4v[:, 3, :])
        V2 = cmb.tile([P, CK], BF16, tag="V2")
        nc.vector.tensor_mul(V2[:, :], V[:, :], r[:, :])
        U = cmb.tile([P, CK], BF16, tag="U")
        nc.vector.tensor_sub(U[:, :], w4v[:, 1, :], V2[:, :])
        Y = cmb.tile([P, CK], BF16, tag="Y")
        nc.vector.tensor_mul(Y[:, :], U[:, :], r[:, :])
        # d2 = Y^2 with the batch-partial sum riding the Act accum port
        d2 = cmb.tile([P, CK], BF16, tag="d2")
        nc.scalar.activation(
            d2[:, :], Y[:, :], AF.Square, accum_out=accT[:, c : c + 1]
        )

        # validity partial: batch sum of K_w per half (>0 iff ref msum-total >0)
        mst_ps = ps.tile([2, CK], F32, tag="mstp")
        for q in range(CK // 512):
            nc.tensor.matmul(
                mst_ps[:, q * 512 : (q + 1) * 512],
                ones_bf[:, :],
                w4v[:, 3, q * 512 : (q + 1) * 512],
                start=True,
                stop=True,
            )
        mst_ev = cmb.tile([2, CK], F32, tag="mst_ev")
        nc.scalar.copy(mst_ev[:, :], mst_ps[:, :])
        nc.sync.dma_start(out=mst_o[:, c * CK : (c + 1) * CK], in_=mst_ev[:, :])


def _build_program():
    nc = bacc.Bacc(
        "TRN2",
        target_bir_lowering=False,
        debug=False,
        enable_asserts=False,
        num_devices=NCORES,
    )
    preds = nc.dram_tensor("predictions", [BL, L, C], F32, kind="ExternalInput")
    tg = nc.dram_tensor("targets", [BL, L], I32, kind="ExternalInput")
    mk = nc.dram_tensor("mask", [BL, L], I32, kind="ExternalInput")
    mst_o = nc.dram_tensor("mst", [2, LH], F32, kind="ExternalOutput")
    acc_o = nc.dram_tensor("acc", [P, NCH], F32, kind="ExternalOutput")

    with tile.TileContext(nc) as tc:
        with (
            tc.tile_pool(name="io", bufs=CFG["prefetch"] + 1) as io,
            tc.tile_pool(name="mid", bufs=3) as mid,
            tc.tile_pool(name="wsp", bufs=3) as wsp,
            tc.tile_pool(name="cmb", bufs=3) as cmb,
            tc.tile_pool(name="const", bufs=1) as const,
            tc.tile_pool(name="ps", bufs=2, space="PSUM") as ps,
        ):
            ones_bf = const.tile([P, 2], BF16)
            nc.vector.memset(ones_bf[:, :], 0.0)
            nc.vector.memset(ones_bf[0:64, 0:1], 1.0)
            nc.vector.memset(ones_bf[64:128, 1:2], 1.0)
            accT = const.tile([P, NCH], F32)

            pools = (io, mid, wsp, cmb, ps)
            _emit_body(nc, pools, ones_bf, accT, preds, tg, mk, mst_o)
            nc.sync.dma_start(out=acc_o[:, :], in_=accT[:, :])

    nc.compile()
    nc.m = get_hw_module(nc.m)
    _split_sync_waits(nc)
    return nc


_NC_CACHE = {}


def _get_nc():
    if "nc" not in _NC_CACHE:
        _NC_CACHE["nc"] = _build_program()
    return _NC_CACHE["nc"]


def run_on_device(predictions, targets, mask, **spmd_kwargs):
    """Shard inputs, run the Bass kernel on 8 cores."""
    nc = _get_nc()
    predictions = np.ascontiguousarray(np.asarray(predictions, np.float32))
    targets = np.ascontiguousarray(np.asarray(targets, np.int32))
    mask = np.ascontiguousarray(np.asarray(mask, np.int32))
    in_maps = []
    for i in range(NCORES):
        sl = slice(i * BL, (i + 1) * BL)
        in_maps.append(
            {
                "predictions": np.ascontiguousarray(predictions[sl]),
                "targets": np.ascontiguousarray(targets[sl]),
                "mask": np.ascontiguousarray(mask[sl]),
            }
        )
    res = run_bass_kernel_spmd(nc, in_maps, core_ids=list(range(NCORES)), **spmd_kwargs)
    return res


def combine_host(results):
    ssd_sum = 0.0
    mst_tot = np.zeros(NW, np.float64)
    for out in results:
        ssd_sum += float(out["acc"].astype(np.float64).sum())
        mst = out["mst"]
        mst_tot += np.concatenate([mst[0], mst[1][: NW - LH]])
    valid = (mst_tot > 0).astype(np.float64)
    cnt = max(valid.sum(), 1.0)
    loss = ssd_sum / B / cnt
    return np.asarray(loss, dtype=np.float32)


def kernel(predictions, targets, mask):
    res = run_on_device(predictions, targets, mask)
    return combine_host(res.results)


if __name__ == "__main__":
    rng = np.random.default_rng(0)
    p = rng.standard_normal((B, L, C), dtype=np.float32)
    t = rng.integers(0, 2, (B, L)).astype(np.int32)
    m = rng.integers(0, 2, (B, L)).astype(np.int32)
    print(kernel(p, t, m))


# revision 29
# speedup vs baseline: 1.1052x; 1.0613x over previous
"""Trainium2 Bass kernel for nn_BoundaryConsistencyLoss.

loss = mean-over-valid-windows of mean-over-batch (pvar - tvar)^2 where
pvar/tvar are masked variances of sigmoid-probs / targets over sliding
windows of 5 along L.

Strategy: pure data parallel over batch (512 = 8 cores x 64 rows).
Per core, SBUF layout [128 partitions = 2 L-halves x 64 batch rows,
free = L-chunk].

Math: with m=mask, t=targets, p=sigmoid(x1-x0), z=(t AND m)+m = m+t*m
(so m=min(z,1), tm=relu(z-1)), define windowed sums via fused
cumsum-custom-ops (one DVE instruction each):
  c_m = cumsum(m), c_G = cumsum(p^2 m - tm), c_H = cumsum(pm - tm),
  c_K = cumsum(pm + tm)
then per window j: X_w[j] = c_X[j+5]-c_X[j], and
  diff = pvar - tvar = r*G_w - r^2*H_w*K_w,  r = 1/(msum+eps)
  d2 = (r*(G_w - r*H_w*K_w))^2
For empty windows (msum=0) all of G_w/H_w/K_w are exactly 0, so d2=0
regardless of r: the clamp max(msum,1) is replaced by a tiny eps bias
inside the reciprocal, and invalid windows self-gate out of the sum.
The total sum of d2 per partition is accumulated for free by the DVE
accum port on the final squaring op; only the per-window validity
indicator (batch sum of K_w, >0 iff the reference's msum total is >0)
goes through a ones-matmul on the tensor engine.  Host sums the 8
cores' partials and finishes the tiny reduction exactly like the
reference.

Engine budget per chunk (the shared SBUF port between DVE-src1 and
GpSimd is the scarce resource): gpsimd runs ONLY SWDGE descriptor
generation; z is assembled during the DMA itself with CCE accum ops
(bypass/min/add over t,m,m); all elementwise work rides the vector
engine; scalar does sigmoid/recip/psum-evac; tensor does the validity
matmul.
"""

import sys

if "/opt/trn_rl_repo" not in sys.path:
    sys.path.insert(0, "/opt/trn_rl_repo")

import numpy as np

import concourse.bass as bass
import concourse.tile as tile
from concourse import bacc, dve_ops, mybir
from concourse.bass_interp import get_hw_module
from concourse.bass_utils import run_bass_kernel_spmd
from concourse.dve_spec import (
    AluOp,
    One,
    Spec,
    Src0,
    Src1,
    _has_src1,
    lower,
    minn,
    relu,
    scan,
    sq,
)
from concourse.dve_uop import DveOpSpec

F32 = mybir.dt.float32
BF16 = mybir.dt.bfloat16
I32 = mybir.dt.int32
AF = mybir.ActivationFunctionType
OP = mybir.AluOpType

NCORES = 8
B, L, C = 512, 16384, 2
BL = B // NCORES          # 64 batch rows per core
LH = L // 2               # 8192: per-half length
W = 5
NW = L - W + 1            # 16380 windows
P = 128

CK = 1024                 # windows computed per chunk
CKH = CK + (W - 1)        # data elements per chunk (halo 4)
NCH = LH // CK
CP = CKH + 4              # c-tile page stride (col 0 is an explicit zero;
                          # 32B-aligned pages keep the windowed-diff reads in
                          # the DVE's 2x dual-port mode)

R_EPS = float(2.0 ** -30)


# --------------------------------------------------------------------------
# custom DVE ops (registered at runtime; sha computed the same way
# DveOp.compile does, so the golden check passes)
# --------------------------------------------------------------------------
def _register_op(name, spec, subdim=False):
    for op in dve_ops.OPS:
        if op.name == name:
            return op
    opcode = dve_ops._CUSTOM_DVE_ROW_BASE + len(dve_ops.OPS)
    shas = {}
    for ver in ("v3", "v4"):
        s = DveOpSpec(
            name=name, opcode=opcode, uops=lower(spec, ver=ver), rd1_en=_has_src1(spec)
        )
        shas[ver] = s.sha(ver)
    op = dve_ops.DveOp(name, spec, subdim=subdim, uops_sha=shas)
    dve_ops.OPS.append(op)
    dve_ops._SUB_OPCODE_FOR_NAME[name] = opcode
    dve_ops.CUSTOM_DVE_SPECS[name] = spec
    return op


def _f32(a):
    return np.asarray(a, np.float32)


def _z_parts(z):
    z = _f32(z)
    return np.minimum(z, 1.0), np.maximum(z - 1.0, 0.0)


def _ref_mscan(in0, in1, s0, s1, imm2):
    return np.cumsum(np.minimum(_f32(in0), 1.0), axis=-1, dtype=np.float32)


def _ref_gscan(in0, in1, s0, s1, imm2):
    m, tm = _z_parts(in1)
    return np.cumsum(_f32(in0) * _f32(in0) * m - tm, axis=-1, dtype=np.float32)


def _ref_hscan(in0, in1, s0, s1, imm2):
    m, tm = _z_parts(in1)
    return np.cumsum(_f32(in0) * m - tm, axis=-1, dtype=np.float32)


def _ref_kscan(in0, in1, s0, s1, imm2):
    m, tm = _z_parts(in1)
    return np.cumsum(_f32(in0) * m + tm, axis=-1, dtype=np.float32)


_m_of_z = minn(Src1, One)
_tm_of_z = relu(Src1 - One)

MSCAN = _register_op(
    "BC2_MSCAN", Spec(body=scan(AluOp.ADD, minn(Src0, One)), reference=_ref_mscan)
)
GSCAN = _register_op(
    "BC2_GSCAN",
    Spec(body=scan(AluOp.ADD, sq(Src0) * _m_of_z - _tm_of_z), reference=_ref_gscan),
)
HSCAN = _register_op(
    "BC2_HSCAN",
    Spec(body=scan(AluOp.ADD, Src0 * _m_of_z - _tm_of_z), reference=_ref_hscan),
)
KSCAN = _register_op(
    "BC2_KSCAN",
    Spec(body=scan(AluOp.ADD, Src0 * _m_of_z + _tm_of_z), reference=_ref_kscan),
)

# d2 = (in0 * in1)^2, with a free running per-partition sum on the accum port
SQMULA = _register_op(
    "BC2_SQMULA",
    Spec(
        body=sq(Src0 * Src1),
        accum=AluOp.ADD,
        reference=lambda in0, in1, s0, s1, imm2: (_f32(in0) * _f32(in1)) ** 2,
    ),
)

# z' = min(t, m) + m fallback (single fused op) if the DMA-accum path is off
ZPRIME = _register_op(
    "BC2_ZPRIME",
    Spec(
        body=minn(Src0, Src1) + Src1,
        reference=lambda in0, in1, s0, s1, imm2: np.minimum(_f32(in0), _f32(in1))
        + _f32(in1),
    ),
)


def _split_sync_waits(nc, max_waits=1):
    """walrus TPB_CTRL codegen rejects >1 explicit sem wait on Drain-class
    instructions; move excess waits onto preceding same-engine no-ops."""
    for fn in nc.m.functions:
        for bb in fn.blocks:
            new_insts = []
            for ins in bb.instructions:
                si = getattr(ins, "sync_info", None)
                waits = list(si.on_wait) if si is not None else []
                if len(waits) > max_waits:
                    extra, keep = waits[:-max_waits], waits[-max_waits:]
                    for j in range(0, len(extra), max_waits):
                        new_insts.append(
                            mybir.InstNoOp(
                                name=f"{ins.name}_wsplit{j}",
                                engine=ins.engine,
                                ins=[],
                                outs=[],
                                sync_info=mybir.SyncInfo(
                                    on_wait=extra[j : j + max_waits], on_update=[]
                                ),
                            )
                        )
                    si.on_wait.clear()
                    si.on_wait.extend(keep)
                new_insts.append(ins)
            bb.instructions = new_insts


CFG = {
    "dsg_engine": "vector",
    "prefetch": 2,         # chunks of DMA issued ahead; deeper prefetch HURTS:
                           # queued SWDGE DMAs round-robin at packet granularity,
                           # so a deep queue delays the oldest chunk's completion
    "m_on_pe": True,       # msum windowed counts via 5 shifted accumulating
                           # identity matmuls on the idle PE instead of
                           # MSCAN + windowed diff on the vector engine
    "tm_dtype": BF16,      # targets/mask load dtype (SWDGE casts i32 on the fly)
}


def _act_scalar(nc, out, in_, func, bias=0.0, scale=1.0):
    """Direct InstActivation emit (bass blocks AF.Reciprocal behind a
    ValueError; our recip inputs are small ints plus eps, well within the
    2e-2 budget)."""
    eng = nc.scalar
    ins = [eng.lower_ap(in_)]
    for val in (bias, scale, 0.0):  # bias, scale, alpha
        ins.append(mybir.ImmediateValue(dtype=mybir.dt.float32, value=val))
    return eng.add_instruction(
        mybir.InstActivation(
            name=nc.get_next_instruction_name(),
            func=func,
            ins=ins,
            outs=[eng.lower_ap(out)],
        )
    )


def _emit_body(nc, pools, ones_bf, iden_bf, accT, preds, tg, mk, mst_o):
    io, mid, wsp, cmb, ps = pools
    TMD = CFG["tm_dtype"]

    def pv(col0, ncols):
        a = preds[:, :, :]
        return bass.AP(tensor=a.tensor, offset=col0, ap=[[L, 2], [2 * L, BL], [1, ncols]])

    def iv(t, col0, ncols):
        a = t[:, :]
        return bass.AP(tensor=a.tensor, offset=col0, ap=[[LH, 2], [L, BL], [1, ncols]])

    pflat = preds.rearrange("b l c -> b (l c)")  # [64, 2*L]

    xps, zts = {}, {}

    def load_chunk(c):
        # ti/mi first: their consumers (ZPRIME, MSCAN) head the chunk's
        # dependency chain, and same-queue SWDGE packets round-robin, so
        # issue order is completion order
        main_z = CK if c == NCH - 1 else CKH
        ti = io.tile([P, CKH], TMD, tag="ti")
        mi = io.tile([P, CKH], TMD, tag="mi")
        zts[c] = (ti, mi)
        nc.gpsimd.dma_start(out=ti[:, :main_z], in_=iv(tg, c * CK, main_z))
        nc.gpsimd.dma_start(out=mi[:, :main_z], in_=iv(mk, c * CK, main_z))
        # predictions piece [128, 2*CKH] f32 (1 MiB + halo)
        xp = io.tile([P, 2 * CKH], F32, tag="xp")
        xps[c] = xp
        main_p = 2 * CK if c == NCH - 1 else 2 * CKH
        nc.gpsimd.dma_start(out=xp[:, :main_p], in_=pv(2 * c * CK, main_p))
        if c == NCH - 1:
            # h=0 rows wrap into the start of the second half; h=1 rows are
            # past the end of L and read as zero
            nc.sync.dma_start(out=xp[0:64, 2 * CK :], in_=pflat[:, 2 * LH : 2 * LH + 8])
            nc.vector.memset(xp[64:128, 2 * CK :], 0.0)
            nc.gpsimd.dma_start(out=ti[0:64, CK:], in_=tg[:, LH : LH + 4])
            nc.gpsimd.dma_start(out=mi[0:64, CK:], in_=mk[:, LH : LH + 4])
            nc.vector.memset(ti[64:128, CK:], 0)
            nc.vector.memset(mi[64:128, CK:], 0)

    for c in range(min(CFG["prefetch"], NCH)):
        load_chunk(c)

    for c in range(NCH):
        if c + CFG["prefetch"] < NCH:
            load_chunk(c + CFG["prefetch"])
        xp = xps.pop(c)
        ti, mi = zts.pop(c)
        zt = mid.tile([P, CKH], F32, tag="zt")
        nc.vector._custom_dve(ZPRIME, out=zt[:, :], in0=ti[:, :], in1=mi[:, :])

        xvv = xp.rearrange("p (l two) -> p l two", two=2)
        dsg = mid.tile([P, CKH], F32, tag="dsg")
        getattr(nc, CFG["dsg_engine"]).tensor_sub(
            dsg[:, :], xvv[:, :, 1], xvv[:, :, 0]
        )
        pp = mid.tile([P, CKH], F32, tag="pp")
        nc.scalar.activation(pp[:, :], dsg[:, :], AF.Sigmoid)

        # fused cumsums, one separate tile per stream (wsub inputs at a tile
        # base reliably hit the DVE 2x dual-port mode); col 0 is an explicit
        # zero so X_w[j] = c[j+5] - c[j] holds for j in [0, CK)
        streams = [("g", GSCAN), ("h", HSCAN), ("k", KSCAN)]
        if not CFG["m_on_pe"]:
            streams = [("m", MSCAN)] + streams
        cts, wts = {}, {}
        for nm, op_ in streams:
            ct = wsp.tile([P, CP], F32, tag=f"c_{nm}")
            cts[nm] = ct
            nc.vector.memset(ct[:, 0:1], 0.0)
            if op_ is MSCAN:
                nc.vector._custom_dve(op_, out=ct[:, 1 : 1 + CKH], in0=mi[:, :])
            else:
                nc.vector._custom_dve(
                    op_, out=ct[:, 1 : 1 + CKH], in0=pp[:, :], in1=zt[:, :]
                )
        for nm, _ in streams:
            wt = cmb.tile([P, CK], BF16, tag=f"w_{nm}")
            wts[nm] = wt
            nc.vector.tensor_sub(
                wt[:, :], cts[nm][:, 5 : 5 + CK], cts[nm][:, 0:CK]
            )

        # r = 1/(msum + eps) on the idle Act engine, bf16 out
        r = cmb.tile([P, CK], BF16, tag="r")
        if CFG["m_on_pe"]:
            # msum via 5 shifted accumulating identity matmuls per 512-half
            for q in range(CK // 512):
                m_ps = ps.tile([P, 512], F32, tag=f"mps{q}")
                for i in range(W):
                    nc.tensor.matmul(
                        m_ps[:, :],
                        iden_bf[:, :],
                        mi[:, q * 512 + i : q * 512 + i + 512],
                        start=(i == 0),
                        stop=(i == W - 1),
                    )
                _act_scalar(
                    nc,
                    r[:, q * 512 : (q + 1) * 512],
                    m_ps[:, :],
                    AF.Reciprocal,
                    bias=R_EPS,
                )
        else:
            _act_scalar(nc, r[:, :], wts["m"][:, :], AF.Reciprocal, bias=R_EPS)

        V = cmb.tile([P, CK], BF16, tag="V")
        nc.vector.tensor_mul(V[:, :], wts["h"][:, :], wts["k"][:, :])
        V2 = cmb.tile([P, CK], BF16, tag="V2")
        nc.vector.tensor_mul(V2[:, :], V[:, :], r[:, :])
        U = cmb.tile([P, CK], BF16, tag="U")
        nc.vector.tensor_sub(U[:, :], wts["g"][:, :], V2[:, :])
        Y = cmb.tile([P, CK], BF16, tag="Y")
        nc.vector.tensor_mul(Y[:, :], U[:, :], r[:, :])
        # d2 = Y^2 with the batch-partial sum riding the Act accum port
        d2 = cmb.tile([P, CK], BF16, tag="d2")
        nc.scalar.activation(
            d2[:, :], Y[:, :], AF.Square, accum_out=accT[:, c : c + 1]
        )

        # validity partial: batch sum of K_w per half (>0 iff ref msum-total >0)
        mst_ps = ps.tile([2, CK], F32, tag="mstp")
        for q in range(CK // 512):
            nc.tensor.matmul(
                mst_ps[:, q * 512 : (q + 1) * 512],
                ones_bf[:, :],
                wts["k"][:, q * 512 : (q + 1) * 512],
                start=True,
                stop=True,
            )
        mst_ev = cmb.tile([2, CK], F32, tag="mst_ev")
        nc.scalar.copy(mst_ev[:, :], mst_ps[:, :])
        nc.sync.dma_start(out=mst_o[:, c * CK : (c + 1) * CK], in_=mst_ev[:, :])


def _build_program():
    nc = bacc.Bacc(
        "TRN2",
        target_bir_lowering=False,
        debug=False,
        enable_asserts=False,
        num_devices=NCORES,
    )
    preds = nc.dram_tensor("predictions", [BL, L, C], F32, kind="ExternalInput")
    tg = nc.dram_tensor("targets", [BL, L], I32, kind="ExternalInput")
    mk = nc.dram_tensor("mask", [BL, L], I32, kind="ExternalInput")
    mst_o = nc.dram_tensor("mst", [2, LH], F32, kind="ExternalOutput")
    acc_o = nc.dram_tensor("acc", [P, NCH], F32, kind="ExternalOutput")

    with tile.TileContext(nc) as tc:
        with (
            tc.tile_pool(name="io", bufs=CFG["prefetch"] + 1) as io,
            tc.tile_pool(name="mid", bufs=3) as mid,
            tc.tile_pool(name="wsp", bufs=3) as wsp,
            tc.tile_pool(name="cmb", bufs=3) as cmb,
            tc.tile_pool(name="const", bufs=1) as const,
            tc.tile_pool(name="ps", bufs=2, space="PSUM") as ps,
        ):
            ones_bf = const.tile([P, 2], BF16)
            nc.vector.memset(ones_bf[:, :], 0.0)
            nc.vector.memset(ones_bf[0:64, 0:1], 1.0)
            nc.vector.memset(ones_bf[64:128, 1:2], 1.0)
            accT = const.tile([P, NCH], F32)
            iden_bf = None
            if CFG["m_on_pe"]:
                # identity weights: iota(j - p) == 0
                iden_bf = const.tile([P, P], BF16)
                scr = const.tile([P, P], I32)
                nc.gpsimd.iota(scr[:, :], pattern=[[1, P]], base=0, channel_multiplier=-1)
                nc.vector.tensor_scalar(
                    iden_bf[:, :], scr[:, :], 0, None, op0=OP.is_equal
                )

            pools = (io, mid, wsp, cmb, ps)
            _emit_body(nc, pools, ones_bf, iden_bf, accT, preds, tg, mk, mst_o)
            nc.sync.dma_start(out=acc_o[:, :], in_=accT[:, :])

    nc.compile()
    nc.m = get_hw_module(nc.m)
    _split_sync_waits(nc)
    return nc


_NC_CACHE = {}


def _get_nc():
    if "nc" not in _NC_CACHE:
        _NC_CACHE["nc"] = _build_program()
    return _NC_CACHE["nc"]


def run_on_device(predictions, targets, mask, **spmd_kwargs):
    """Shard inputs, run the Bass kernel on 8 cores."""
    nc = _get_nc()
    predictions = np.ascontiguousarray(np.asarray(predictions, np.float32))
    targets = np.ascontiguousarray(np.asarray(targets, np.int32))
    mask = np.ascontiguousarray(np.asarray(mask, np.int32))
    in_maps = []
    for i in range(NCORES):
        sl = slice(i * BL, (i + 1) * BL)
        in_maps.append(
            {
                "predictions": np.ascontiguousarray(predictions[sl]),
                "targets": np.ascontiguousarray(targets[sl]),
                "mask": np.ascontiguousarray(mask[sl]),
            }
        )
    res = run_bass_kernel_spmd(nc, in_maps, core_ids=list(range(NCORES)), **spmd_kwargs)
    return res


def combine_host(results):
    ssd_sum = 0.0
    mst_tot = np.zeros(NW, np.float64)
    for out in results:
        ssd_sum += float(out["acc"].astype(np.float64).sum())
        mst = out["mst"]
        mst_tot += np.concatenate([mst[0], mst[1][: NW - LH]])
    valid = (mst_tot > 0).astype(np.float64)
    cnt = max(valid.sum(), 1.0)
    loss = ssd_sum / B / cnt
    return np.asarray(loss, dtype=np.float32)


def kernel(predictions, targets, mask):
    res = run_on_device(predictions, targets, mask)
    return combine_host(res.results)


if __name__ == "__main__":
    rng = np.random.default_rng(0)
    p = rng.standard_normal((B, L, C), dtype=np.float32)
    t = rng.integers(0, 2, (B, L)).astype(np.int32)
    m = rng.integers(0, 2, (B, L)).astype(np.int32)
    print(kernel(p, t, m))
